# revision 18
# baseline (speedup 1.0000x reference)
"""Multi-head attention (B=2,S=2048,E=1024,H=16,D=64) on 8 trn2 NeuronCores.

Sharding: cores split into 2 batch groups x 4 head-group cores.
Core c: batch b=c//4, head group g=c%4 (heads 4g..4g+3, i.e. 256 d-cols).

All matmul operands are bf16 (PSUM accumulation stays fp32).  Dataflow is
"transposed" (host feeds x^T so contractions sit on partitions):
  q^T/k^T = W^T-slice matmuls producing [d, tok] tiles, v in [tok, d] layout
  with a ones column (softmax denominators ride the AV matmul), scores
  computed as S^T = [keys, q] so AV needs no transpose, exp without
  max-subtraction (scores are tiny for this problem; verified host-side).

Schedule: per 512-query chunk c, a wave of {proj_k(c), proj_q(c),
proj_v(kt 4c..4c+3), attn(c)} so the scalar-engine exp stream starts early;
each (chunk, head-pair) attn output is AllGather'd (4-rank group) while
later chunks compute; out-projection consumes the gathered pieces at the
end (pair-0 gather of the last chunk overlaps pair-1's attention).
bv is folded host-side into bo (softmax rows sum to 1 => attention output
== attn @ (x Wv) + bv exactly), so the kernel never touches bv.
"""

import os
import sys

for _p in ("/opt/trn_rl_repo", "/root/.axon_site/_ro/trn_rl_repo"):
    if os.path.isdir(_p) and _p not in sys.path:
        sys.path.insert(0, _p)

import ml_dtypes
import numpy as np

import concourse.bacc as bacc
import concourse.bass as bass
import concourse.mybir as mybir
import concourse.tile as tile
from concourse.bass import ds, ts
from concourse.bass_utils import run_bass_kernel_spmd

F32 = mybir.dt.float32
BF16 = mybir.dt.bfloat16
NPBF16 = ml_dtypes.bfloat16

B, S, E, H, D = 2, 2048, 1024, 16, 64
NCORES = 8
HG = 4                 # head-group cores per batch
HPC = H // HG          # heads per core (4)
DPC = HPC * D          # d-cols per core (256)
NPAIR = DPC // 128     # 128-row head pairs per core (2)
TOK = S                # tokens per core's batch
QCH = 512              # query chunk (matmul moving dim)
NCH = TOK // QCH       # chunks (4)
KT = 128               # key tile
NKT = TOK // KT        # key tiles (16)
NE = E // 128          # contraction tiles (8)
NEG = -30000.0
INV_D = 1.0 / float(D)  # folded double scaling (1/64)

AluOp = mybir.AluOpType
ActFn = mybir.ActivationFunctionType


def build_nc():
    nc = bacc.Bacc(None, target_bir_lowering=False, num_devices=NCORES)

    # --- I/O ---
    xq_t = nc.dram_tensor("xq_t", [E, TOK], BF16, kind="ExternalInput")
    xk_t = nc.dram_tensor("xk_t", [E, TOK], BF16, kind="ExternalInput")
    xv_t = nc.dram_tensor("xv_t", [E, TOK], BF16, kind="ExternalInput")
    wq_d = nc.dram_tensor("wq", [E, DPC], BF16, kind="ExternalInput")
    wk_d = nc.dram_tensor("wk", [E, DPC], BF16, kind="ExternalInput")
    wv_d = nc.dram_tensor("wv", [E, DPC], BF16, kind="ExternalInput")
    wo_d = nc.dram_tensor("wo", [E, DPC], BF16, kind="ExternalInput")
    bq_d = nc.dram_tensor("bq_p", [128, NPAIR], F32, kind="ExternalInput")
    bk_d = nc.dram_tensor("bk_p", [128, NPAIR], F32, kind="ExternalInput")
    bo_d = nc.dram_tensor("bo_p", [128, NPAIR], F32, kind="ExternalInput")
    mask_d = nc.dram_tensor("maskadd", [128, 4, QCH], F32, kind="ExternalInput")
    out_d = nc.dram_tensor("out_t", [DPC, TOK], F32, kind="ExternalOutput")

    # per-chunk collective buffers (DRAM); 4-rank groups.  One gather per
    # chunk: collectives serialize on the CC stream with ~10us fixed cost
    # each, so fewer/larger is better (only the last one is ever exposed).
    agin = [nc.dram_tensor(f"agin{c}", [DPC, QCH], BF16) for c in range(NCH)]
    agout = [nc.dram_tensor(f"agout{c}", [HG * DPC, QCH], BF16) for c in range(NCH)]

    groups = [[g * HG + r for r in range(HG)] for g in range(NCORES // HG)]

    with tile.TileContext(nc) as tc:
        import contextlib

        with contextlib.ExitStack() as ctx:
            p_const = ctx.enter_context(tc.tile_pool(name="const", bufs=1))
            p_x = ctx.enter_context(tc.tile_pool(name="xin", bufs=24))
            p_ag = ctx.enter_context(tc.tile_pool(name="agb", bufs=10))
            p_pers = ctx.enter_context(tc.tile_pool(name="pers", bufs=2))
            p_v = ctx.enter_context(tc.tile_pool(name="vbuf", bufs=1))
            p_exp = ctx.enter_context(tc.tile_pool(name="expb", bufs=4))
            p_sm = ctx.enter_context(tc.tile_pool(name="small", bufs=4))
            p_out = ctx.enter_context(tc.tile_pool(name="outs", bufs=2))
            p_ps = ctx.enter_context(tc.tile_pool(name="ps", bufs=4, space="PSUM"))
            p_sc = ctx.enter_context(tc.tile_pool(name="sc", bufs=2, space="PSUM"))

            # --- constants (DMA order matters: k/q weights + biases first,
            # then x quarter-waves interleave below) ---
            wq_sb = p_const.tile([128, NE, DPC], BF16, name="wq_sb")
            wk_sb = p_const.tile([128, NE, DPC], BF16, name="wk_sb")
            wv_sb = p_const.tile([128, NE, DPC], BF16, name="wv_sb")
            wo_sb = p_const.tile([128, NE, DPC], BF16, name="wo_sb")
            bq_sb = p_const.tile([128, NPAIR], F32, name="bq_sb")
            bk_sb = p_const.tile([128, NPAIR], F32, name="bk_sb")
            bo_sb = p_const.tile([128, NPAIR], F32, name="bo_sb")
            mask_sb = p_const.tile([128, 4, QCH], F32, name="mask_sb")

            # persistent activations
            qT = [p_pers.tile([128, TOK], BF16, name="qT", tag="qT") for _ in range(NPAIR)]
            kT = [p_pers.tile([128, TOK], BF16, name="kT", tag="kT") for _ in range(NPAIR)]
            # v: [tok_part, kt, head, 128] ; col 0 = ones (softmax denom row),
            # cols 1..63 zero pad (engine APs can only start at partition 0/64
            # and span 128/64), cols 64..127 = v
            v_sb = p_v.tile([128, NKT, HPC, 128], BF16, name="v_sb")
            nc.any.memset(v_sb[:, :, :, 0:64], 0.0)
            nc.any.memset(v_sb[:, :, :, 0:1], 1.0)

            # --- x staging: [128, TOK] per e-tile, DMA'd in column quarters
            # (chunk-need order) so wave c never waits on later columns ---
            def stage_x(tag):
                return [
                    p_x.tile([128, TOK], BF16, name=f"x_{tag}", tag="x")
                    for _ in range(NE)
                ]

            xk_tiles = stage_x("k")
            xq_tiles = stage_x("q")
            xv_tiles = stage_x("v")
            def load_w(w_sb, w_d):
                nc.sync.dma_start(
                    out=w_sb[:, :, :],
                    in_=w_d.ap().rearrange("(e p) n -> p e n", p=128),
                )

            def load_x_quarter(tiles, x_d, quarter):
                for e in range(NE):
                    nc.sync.dma_start(
                        out=tiles[e][:, ts(quarter, QCH)],
                        in_=x_d[ts(e, 128), ts(quarter, QCH)],
                    )

            # startup-critical order: each weight immediately followed by the
            # x columns its first chain consumes.  Quarters 1-3 are issued
            # inside the schedule below so latency-critical attn/gather DMAs
            # are not stuck behind the bulk-x ring backlog.
            load_w(wk_sb, wk_d)
            load_x_quarter(xk_tiles, xk_t, 0)
            nc.sync.dma_start(out=bk_sb[:, :], in_=bk_d[:, :])
            load_x_quarter(xk_tiles, xk_t, 1)
            load_w(wv_sb, wv_d)
            load_x_quarter(xv_tiles, xv_t, 0)
            load_w(wq_sb, wq_d)
            load_x_quarter(xq_tiles, xq_t, 1)
            nc.sync.dma_start(out=bq_sb[:, :], in_=bq_d[:, :])
            load_x_quarter(xv_tiles, xv_t, 1)
            nc.sync.dma_start(out=mask_sb[:, :, :], in_=mask_d[:, :, :])
            load_w(wo_sb, wo_d)
            nc.sync.dma_start(out=bo_sb[:, :], in_=bo_d[:, :])

            def load_x_wave(quarter):
                # bulk columns for projection wave `quarter`
                load_x_quarter(xk_tiles, xk_t, quarter)
                load_x_quarter(xv_tiles, xv_t, quarter)
                load_x_quarter(xq_tiles, xq_t, quarter)

            # --- building blocks ---
            def proj_wave(which, c):
                # k or q projection for chunk c (both pairs)
                for w_sb_, xtiles, dst, bias, scale in (
                    (
                        (wk_sb, xk_tiles, kT, bk_sb, None),
                        (wq_sb, xq_tiles, qT, bq_sb, INV_D),
                    )[which : which + 1]
                ):
                    for p in range(NPAIR):
                        ps = p_ps.tile([128, QCH], F32, name="ps_kq", tag="ps")
                        for e in range(NE):
                            nc.tensor.matmul(
                                ps[:, :],
                                w_sb_[:, e, ts(p, 128)],
                                xtiles[e][:, ts(c, QCH)],
                                start=(e == 0),
                                stop=(e == NE - 1),
                            )
                        if scale is None:
                            nc.vector.tensor_scalar(
                                out=dst[p][:, ts(c, QCH)],
                                in0=ps[:, :],
                                scalar1=bias[:, p : p + 1],
                                scalar2=None,
                                op0=AluOp.add,
                            )
                        else:
                            nc.vector.tensor_scalar(
                                out=dst[p][:, ts(c, QCH)],
                                in0=ps[:, :],
                                scalar1=bias[:, p : p + 1],
                                scalar2=scale,
                                op0=AluOp.add,
                                op1=AluOp.mult,
                            )

            def v_wave(c):
                # v projection for key tiles 4c..4c+3 ([tok,d] layout)
                for m in range(4 * c, 4 * c + 4):
                    ps = p_ps.tile([128, QCH], F32, name="ps_v", tag="ps")
                    for e in range(NE):
                        nc.tensor.matmul(
                            ps[:, 0:DPC],
                            xv_tiles[e][:, ts(m, 128)],
                            wv_sb[:, e, :],
                            start=(e == 0),
                            stop=(e == NE - 1),
                        )
                    nc.vector.tensor_copy(
                        out=v_sb[:, m, :, 64:128],
                        in_=ps[:, 0:DPC].rearrange("p (h d) -> p h d", h=HPC),
                    )

            def attn_chunk(c):
                for p in range(NPAIR):
                    av = [
                        p_ps.tile([128, QCH], F32, name="ps_av", tag="ps")
                        for _ in range(2)
                    ]
                    nkt_c = 4 * (c + 1)

                    def issue_av(kt, ex, qo):
                        for h in range(2):
                            nc.tensor.matmul(
                                av[h][:, qo:QCH],
                                v_sb[:, kt, p * 2 + h, 0:128],
                                ex[:, h, qo:QCH],
                                start=(kt == 0),
                                stop=(kt == nkt_c - 1),
                            )

                    pend = None
                    for kt in range(nkt_c):
                        # diagonal tile at offset o: queries < 128*o see only
                        # masked keys (exp == 0, contributes nothing) -- skip
                        # those columns in scores/mask/exp/AV entirely
                        o = kt - 4 * c
                        qo = 128 * o if o >= 0 else 0
                        qn = QCH - qo
                        sc = p_sc.tile([128, 2, QCH], F32, name="sc", tag="sc")
                        for h in range(2):
                            nc.tensor.matmul(
                                sc[:, h, qo:QCH],
                                kT[p][ds(h * 64, 64), ts(kt, 128)],
                                qT[p][ds(h * 64, 64), ds(c * QCH + qo, qn)],
                                start=True,
                                stop=True,
                                tile_position=(h * 64, 0),
                            )
                        # AV of the previous key tile goes to the PE queue
                        # here so the engine has work while exp(kt) runs
                        if pend is not None:
                            issue_av(*pend)
                        if o >= 0:
                            nc.vector.tensor_tensor(
                                out=sc[:, :, qo:QCH],
                                in0=sc[:, :, qo:QCH],
                                in1=mask_sb[:, o : o + 1, qo:QCH]
                                .broadcast_to((128, 2, qn)),
                                op=AluOp.add,
                            )
                        ex = p_exp.tile([128, 2, QCH], BF16, name="ex", tag="ex")
                        nc.scalar.activation(
                            ex[:, :, qo:QCH], sc[:, :, qo:QCH], ActFn.Exp
                        )
                        pend = (kt, ex, qo)
                    issue_av(*pend)

                    # normalize + ship to the collective input buffer
                    for h in range(2):
                        den = p_sm.tile([1, QCH], F32, name="den", tag="den")
                        nc.vector.tensor_copy(out=den[0:1, :], in_=av[h][0:1, :])
                        rcp = p_sm.tile([1, QCH], F32, name="rcp", tag="rcp")
                        nc.vector.reciprocal_approx_fast(rcp[0:1, :], den[0:1, :])
                        rep = p_sm.tile([128, QCH], F32, name="rep", tag="rep")
                        nc.gpsimd.partition_broadcast(rep[0:128, :], rcp[0:1, :])
                        an = p_sm.tile([128, QCH], BF16, name="an", tag="an")
                        nc.vector.tensor_tensor(
                            out=an[64:128, :],
                            in0=av[h][64:128, :],
                            in1=rep[64:128, :],
                            op=AluOp.mult,
                        )
                        nc.sync.dma_start(
                            out=agin[c][ds(p * 128 + h * 64, 64), :],
                            in_=an[64:128, :],
                        )
                # gather this chunk's attn output across the 4-rank group
                nc.gpsimd.collective_compute(
                    "AllGather",
                    AluOp.bypass,
                    replica_groups=groups,
                    ins=[agin[c].ap()],
                    outs=[agout[c].ap()],
                )

            def outproj_chunk(c):
                pso = [
                    p_ps.tile([128, QCH], F32, name="pso", tag="ps")
                    for _ in range(NPAIR)
                ]
                for e in range(NE):
                    ag_sb = p_ag.tile([128, QCH], BF16, name="ag_sb", tag="ag")
                    nc.sync.dma_start(
                        out=ag_sb[:, :], in_=agout[c][ts(e, 128), :]
                    )
                    for po in range(NPAIR):
                        nc.tensor.matmul(
                            pso[po][:, :],
                            wo_sb[:, e, ts(po, 128)],
                            ag_sb[:, :],
                            start=(e == 0),
                            stop=(e == NE - 1),
                        )
                for po in range(NPAIR):
                    ot = p_out.tile([128, QCH], F32, name="ot", tag="ot")
                    nc.vector.tensor_scalar(
                        out=ot[:, :],
                        in0=pso[po][:, :],
                        scalar1=bo_sb[:, po : po + 1],
                        scalar2=None,
                        op0=AluOp.add,
                    )
                    nc.sync.dma_start(
                        out=out_d[ts(po, 128), ts(c, QCH)], in_=ot[:, :]
                    )

            # --- schedule ---
            # Projection waves ascending (chunk c unlocks attn kt<=4c+3);
            # attention pieces ordered [c1, c2, c3, c0] so the LAST piece is
            # the smallest chunk (short serial tail before the final gather);
            # each out-projection is interleaved one attn chunk later so its
            # gather has landed by the time the PE reaches it.  q(0) is only
            # needed by the final attn piece and is deferred, which lets the
            # exp stream start right after k/v waves 0-1 + q(1).
            with nc.named_scope("wave0"):
                proj_wave(0, 0)
                v_wave(0)
            with nc.named_scope("wave1"):
                proj_wave(0, 1)
                v_wave(1)
                proj_wave(1, 1)
            with nc.named_scope("attn1"):
                attn_chunk(1)
            with nc.named_scope("wave2"):
                load_x_wave(2)
                proj_wave(0, 2)
                v_wave(2)
                proj_wave(1, 2)
            with nc.named_scope("attn2"):
                attn_chunk(2)
            with nc.named_scope("oproj1"):
                outproj_chunk(1)
            with nc.named_scope("wave3"):
                load_x_wave(3)
                load_x_quarter(xq_tiles, xq_t, 0)
                proj_wave(0, 3)
                v_wave(3)
                proj_wave(1, 3)
            with nc.named_scope("attn3"):
                attn_chunk(3)
            with nc.named_scope("oproj2"):
                outproj_chunk(2)
            with nc.named_scope("wave0q"):
                proj_wave(1, 0)
            with nc.named_scope("attn0"):
                attn_chunk(0)
            with nc.named_scope("oproj3"):
                outproj_chunk(3)
            with nc.named_scope("oproj0"):
                outproj_chunk(0)

    nc.compile()
    return nc


_NC_CACHE = None


def _get_nc():
    global _NC_CACHE
    if _NC_CACHE is None:
        _NC_CACHE = build_nc()
    return _NC_CACHE


def _prep_in_maps(query, key, value, Wq, Wk, Wv, Wo, bq, bk, bv, bo, attn_mask):
    query = np.asarray(query, np.float32).reshape(B, S, E)
    key = np.asarray(key, np.float32).reshape(B, S, E)
    value = np.asarray(value, np.float32).reshape(B, S, E)
    m = np.asarray(attn_mask, bool)
    expect = np.triu(np.ones((S, S), bool), k=1)
    if not np.array_equal(m, expect):
        raise ValueError("kernel specialized for causal attn_mask")
    # additive mask for the 4 key-tile offsets inside a diagonal 512-block:
    # maskadd[p, o, f] = NEG where key=128*o+p is masked for query f
    sub = m[:QCH, :QCH]  # [q, k]
    maskadd = np.where(sub.T.reshape(4, 128, QCH), np.float32(NEG), np.float32(0.0))
    maskadd = np.ascontiguousarray(maskadd.transpose(1, 0, 2))  # [128, 4, 512]

    # softmax rows sum to 1, so attn_out = attn @ (x Wv) + bv exactly;
    # fold bv through the out-projection into bo.
    bo_eff = (
        np.asarray(bv, np.float64) @ np.asarray(Wo, np.float64)
        + np.asarray(bo, np.float64)
    ).astype(np.float32)

    xT = [
        [np.ascontiguousarray(t[b].T).astype(NPBF16) for t in (query, key, value)]
        for b in range(B)
    ]

    in_maps = []
    for c in range(NCORES):
        b, g = divmod(c, HG)
        cs = slice(DPC * g, DPC * (g + 1))
        in_maps.append(
            {
                "xq_t": xT[b][0],
                "xk_t": xT[b][1],
                "xv_t": xT[b][2],
                "wq": np.ascontiguousarray(Wq[:, cs]).astype(NPBF16),
                "wk": np.ascontiguousarray(Wk[:, cs]).astype(NPBF16),
                "wv": np.ascontiguousarray(Wv[:, cs]).astype(NPBF16),
                "wo": np.ascontiguousarray(Wo[:, cs]).astype(NPBF16),
                "bq_p": np.ascontiguousarray(
                    np.asarray(bq, np.float32)[cs].reshape(NPAIR, 128).T
                ),
                "bk_p": np.ascontiguousarray(
                    np.asarray(bk, np.float32)[cs].reshape(NPAIR, 128).T
                ),
                "bo_p": np.ascontiguousarray(
                    bo_eff[cs].reshape(NPAIR, 128).T
                ),
                "maskadd": maskadd,
            }
        )
    return in_maps


def _assemble(results):
    outs = []
    for b in range(B):
        cols = [results[b * HG + g]["out_t"] for g in range(HG)]
        outs.append(np.concatenate(cols, axis=0).T)  # [TOK, E]
    return np.ascontiguousarray(np.stack(outs, axis=0).astype(np.float32))


def kernel(**inputs):
    nc = _get_nc()
    in_maps = _prep_in_maps(**inputs)
    res = run_bass_kernel_spmd(nc, in_maps, core_ids=list(range(NCORES)))
    return _assemble(res.results)


if __name__ == "__main__":
    import reference

    inputs = {k: np.asarray(v) for k, v in reference.setup_inputs().items()}
    out = kernel(**inputs)
    exp = np.asarray(reference.reference(**reference.setup_inputs()))
    err = np.abs(out - exp).max() / np.abs(exp).max()
    print("rel err:", err)


# revision 20
# speedup vs baseline: 1.0132x; 1.0132x over previous
"""Multi-head attention (B=2,S=2048,E=1024,H=16,D=64) on 8 trn2 NeuronCores.

Sharding: cores split into 2 batch groups x 4 head-group cores.
Core c: batch b=c//4, head group g=c%4 (heads 4g..4g+3, i.e. 256 d-cols).

All matmul operands are bf16 (PSUM accumulation stays fp32).  Dataflow is
"transposed" (host feeds x^T so contractions sit on partitions):
  q^T/k^T = W^T-slice matmuls producing [d, tok] tiles, v in [tok, d] layout
  with a ones column (softmax denominators ride the AV matmul), scores
  computed as S^T = [keys, q] so AV needs no transpose, exp without
  max-subtraction (scores are tiny for this problem; verified host-side).

Schedule: per 512-query chunk c, a wave of {proj_k(c), proj_q(c),
proj_v(kt 4c..4c+3), attn(c)} so the scalar-engine exp stream starts early;
each (chunk, head-pair) attn output is AllGather'd (4-rank group) while
later chunks compute; out-projection consumes the gathered pieces at the
end (pair-0 gather of the last chunk overlaps pair-1's attention).
bv is folded host-side into bo (softmax rows sum to 1 => attention output
== attn @ (x Wv) + bv exactly), so the kernel never touches bv.
"""

import os
import sys

for _p in ("/opt/trn_rl_repo", "/root/.axon_site/_ro/trn_rl_repo"):
    if os.path.isdir(_p) and _p not in sys.path:
        sys.path.insert(0, _p)

import ml_dtypes
import numpy as np

import concourse.bacc as bacc
import concourse.bass as bass
import concourse.mybir as mybir
import concourse.tile as tile
from concourse.bass import ds, ts
from concourse.bass_utils import run_bass_kernel_spmd

F32 = mybir.dt.float32
BF16 = mybir.dt.bfloat16
NPBF16 = ml_dtypes.bfloat16

B, S, E, H, D = 2, 2048, 1024, 16, 64
NCORES = 8
HG = 4                 # head-group cores per batch
HPC = H // HG          # heads per core (4)
DPC = HPC * D          # d-cols per core (256)
NPAIR = DPC // 128     # 128-row head pairs per core (2)
TOK = S                # tokens per core's batch
QCH = 512              # query chunk (matmul moving dim)
NCH = TOK // QCH       # chunks (4)
KT = 128               # key tile
NKT = TOK // KT        # key tiles (16)
NE = E // 128          # contraction tiles (8)
NEG = -30000.0
INV_D = 1.0 / float(D)  # folded double scaling (1/64)

AluOp = mybir.AluOpType
ActFn = mybir.ActivationFunctionType


def build_nc():
    nc = bacc.Bacc(None, target_bir_lowering=False, num_devices=NCORES)

    # --- I/O ---
    xq_t = nc.dram_tensor("xq_t", [E, TOK], BF16, kind="ExternalInput")
    xk_t = nc.dram_tensor("xk_t", [E, TOK], BF16, kind="ExternalInput")
    xv_t = nc.dram_tensor("xv_t", [E, TOK], BF16, kind="ExternalInput")
    wq_d = nc.dram_tensor("wq", [E, DPC], BF16, kind="ExternalInput")
    wk_d = nc.dram_tensor("wk", [E, DPC], BF16, kind="ExternalInput")
    wv_d = nc.dram_tensor("wv", [E, DPC], BF16, kind="ExternalInput")
    wo_d = nc.dram_tensor("wo", [E, DPC], BF16, kind="ExternalInput")
    bq_d = nc.dram_tensor("bq_p", [128, NPAIR], F32, kind="ExternalInput")
    bk_d = nc.dram_tensor("bk_p", [128, NPAIR], F32, kind="ExternalInput")
    bo_d = nc.dram_tensor("bo_p", [128, NPAIR], F32, kind="ExternalInput")
    mask_d = nc.dram_tensor("maskadd", [128, 4, QCH], F32, kind="ExternalInput")
    out_d = nc.dram_tensor("out_t", [DPC, TOK], F32, kind="ExternalOutput")

    # per-chunk collective buffers (DRAM); 4-rank groups.  One gather per
    # chunk: collectives serialize on the CC stream with ~10us fixed cost
    # each, so fewer/larger is better (only the last one is ever exposed).
    agin = [nc.dram_tensor(f"agin{c}", [DPC, QCH], BF16) for c in range(NCH)]
    agout = [nc.dram_tensor(f"agout{c}", [HG * DPC, QCH], BF16) for c in range(NCH)]

    groups = [[g * HG + r for r in range(HG)] for g in range(NCORES // HG)]

    with tile.TileContext(nc) as tc:
        import contextlib

        with contextlib.ExitStack() as ctx:
            p_const = ctx.enter_context(tc.tile_pool(name="const", bufs=1))
            p_x = ctx.enter_context(tc.tile_pool(name="xin", bufs=24))
            p_ag = ctx.enter_context(tc.tile_pool(name="agb", bufs=10))
            p_pers = ctx.enter_context(tc.tile_pool(name="pers", bufs=2))
            p_v = ctx.enter_context(tc.tile_pool(name="vbuf", bufs=1))
            p_exp = ctx.enter_context(tc.tile_pool(name="expb", bufs=4))
            p_sm = ctx.enter_context(tc.tile_pool(name="small", bufs=4))
            p_out = ctx.enter_context(tc.tile_pool(name="outs", bufs=2))
            p_ps = ctx.enter_context(tc.tile_pool(name="ps", bufs=4, space="PSUM"))
            p_sc = ctx.enter_context(tc.tile_pool(name="sc", bufs=2, space="PSUM"))

            # --- constants (DMA order matters: k/q weights + biases first,
            # then x quarter-waves interleave below) ---
            wq_sb = p_const.tile([128, NE, DPC], BF16, name="wq_sb")
            wk_sb = p_const.tile([128, NE, DPC], BF16, name="wk_sb")
            wv_sb = p_const.tile([128, NE, DPC], BF16, name="wv_sb")
            wo_sb = p_const.tile([128, NE, DPC], BF16, name="wo_sb")
            bq_sb = p_const.tile([128, NPAIR], F32, name="bq_sb")
            bk_sb = p_const.tile([128, NPAIR], F32, name="bk_sb")
            bo_sb = p_const.tile([128, NPAIR], F32, name="bo_sb")
            mask_sb = p_const.tile([128, 4, QCH], F32, name="mask_sb")

            # persistent activations
            qT = [p_pers.tile([128, TOK], BF16, name="qT", tag="qT") for _ in range(NPAIR)]
            kT = [p_pers.tile([128, TOK], BF16, name="kT", tag="kT") for _ in range(NPAIR)]
            # v: [tok_part, kt, head, 128] ; col 0 = ones (softmax denom row),
            # cols 1..63 zero pad (engine APs can only start at partition 0/64
            # and span 128/64), cols 64..127 = v
            v_sb = p_v.tile([128, NKT, HPC, 128], BF16, name="v_sb")
            nc.any.memset(v_sb[:, :, :, 0:64], 0.0)
            nc.any.memset(v_sb[:, :, :, 0:1], 1.0)

            # --- x staging: [128, TOK] per e-tile, DMA'd in column quarters
            # (chunk-need order) so wave c never waits on later columns ---
            def stage_x(tag):
                return [
                    p_x.tile([128, TOK], BF16, name=f"x_{tag}", tag="x")
                    for _ in range(NE)
                ]

            xk_tiles = stage_x("k")
            xq_tiles = stage_x("q")
            xv_tiles = stage_x("v")
            def load_w(w_sb, w_d):
                nc.sync.dma_start(
                    out=w_sb[:, :, :],
                    in_=w_d.ap().rearrange("(e p) n -> p e n", p=128),
                )

            def load_x_quarter(tiles, x_d, quarter):
                for e in range(NE):
                    nc.sync.dma_start(
                        out=tiles[e][:, ts(quarter, QCH)],
                        in_=x_d[ts(e, 128), ts(quarter, QCH)],
                    )

            # startup-critical order: each weight immediately followed by the
            # x columns its first chain consumes.  Quarters 1-3 are issued
            # inside the schedule below so latency-critical attn/gather DMAs
            # are not stuck behind the bulk-x ring backlog.
            load_w(wk_sb, wk_d)
            load_x_quarter(xk_tiles, xk_t, 0)
            nc.sync.dma_start(out=bk_sb[:, :], in_=bk_d[:, :])
            load_w(wv_sb, wv_d)
            load_x_quarter(xv_tiles, xv_t, 0)
            load_x_quarter(xk_tiles, xk_t, 1)
            load_w(wq_sb, wq_d)
            load_x_quarter(xq_tiles, xq_t, 1)
            nc.sync.dma_start(out=bq_sb[:, :], in_=bq_d[:, :])
            load_x_quarter(xv_tiles, xv_t, 1)
            nc.sync.dma_start(out=mask_sb[:, :, :], in_=mask_d[:, :, :])
            load_w(wo_sb, wo_d)
            nc.sync.dma_start(out=bo_sb[:, :], in_=bo_d[:, :])

            def load_x_wave(quarter):
                # bulk columns for projection wave `quarter`
                load_x_quarter(xk_tiles, xk_t, quarter)
                load_x_quarter(xv_tiles, xv_t, quarter)
                load_x_quarter(xq_tiles, xq_t, quarter)

            # --- building blocks ---
            def proj_wave(which, c):
                # k or q projection for chunk c (both pairs)
                for w_sb_, xtiles, dst, bias, scale in (
                    (
                        (wk_sb, xk_tiles, kT, bk_sb, None),
                        (wq_sb, xq_tiles, qT, bq_sb, INV_D),
                    )[which : which + 1]
                ):
                    for p in range(NPAIR):
                        ps = p_ps.tile([128, QCH], F32, name="ps_kq", tag="ps")
                        for e in range(NE):
                            nc.tensor.matmul(
                                ps[:, :],
                                w_sb_[:, e, ts(p, 128)],
                                xtiles[e][:, ts(c, QCH)],
                                start=(e == 0),
                                stop=(e == NE - 1),
                            )
                        if scale is None:
                            nc.vector.tensor_scalar(
                                out=dst[p][:, ts(c, QCH)],
                                in0=ps[:, :],
                                scalar1=bias[:, p : p + 1],
                                scalar2=None,
                                op0=AluOp.add,
                            )
                        else:
                            nc.vector.tensor_scalar(
                                out=dst[p][:, ts(c, QCH)],
                                in0=ps[:, :],
                                scalar1=bias[:, p : p + 1],
                                scalar2=scale,
                                op0=AluOp.add,
                                op1=AluOp.mult,
                            )

            def v_wave(c):
                # v projection for key tiles 4c..4c+3 ([tok,d] layout)
                for m in range(4 * c, 4 * c + 4):
                    ps = p_ps.tile([128, QCH], F32, name="ps_v", tag="ps")
                    for e in range(NE):
                        nc.tensor.matmul(
                            ps[:, 0:DPC],
                            xv_tiles[e][:, ts(m, 128)],
                            wv_sb[:, e, :],
                            start=(e == 0),
                            stop=(e == NE - 1),
                        )
                    nc.vector.tensor_copy(
                        out=v_sb[:, m, :, 64:128],
                        in_=ps[:, 0:DPC].rearrange("p (h d) -> p h d", h=HPC),
                    )

            def attn_chunk(c):
                for p in range(NPAIR):
                    av = [
                        p_ps.tile([128, QCH], F32, name="ps_av", tag="ps")
                        for _ in range(2)
                    ]
                    nkt_c = 4 * (c + 1)

                    def issue_av(kt, ex, qo):
                        for h in range(2):
                            nc.tensor.matmul(
                                av[h][:, qo:QCH],
                                v_sb[:, kt, p * 2 + h, 0:128],
                                ex[:, h, qo:QCH],
                                start=(kt == 0),
                                stop=(kt == nkt_c - 1),
                            )

                    pend = None
                    for kt in range(nkt_c):
                        # diagonal tile at offset o: queries < 128*o see only
                        # masked keys (exp == 0, contributes nothing) -- skip
                        # those columns in scores/mask/exp/AV entirely
                        o = kt - 4 * c
                        qo = 128 * o if o >= 0 else 0
                        qn = QCH - qo
                        sc = p_sc.tile([128, 2, QCH], F32, name="sc", tag="sc")
                        for h in range(2):
                            nc.tensor.matmul(
                                sc[:, h, qo:QCH],
                                kT[p][ds(h * 64, 64), ts(kt, 128)],
                                qT[p][ds(h * 64, 64), ds(c * QCH + qo, qn)],
                                start=True,
                                stop=True,
                                tile_position=(h * 64, 0),
                            )
                        # AV of the previous key tile goes to the PE queue
                        # here so the engine has work while exp(kt) runs
                        if pend is not None:
                            issue_av(*pend)
                        if o >= 0:
                            nc.vector.tensor_tensor(
                                out=sc[:, :, qo:QCH],
                                in0=sc[:, :, qo:QCH],
                                in1=mask_sb[:, o : o + 1, qo:QCH]
                                .broadcast_to((128, 2, qn)),
                                op=AluOp.add,
                            )
                        ex = p_exp.tile([128, 2, QCH], BF16, name="ex", tag="ex")
                        nc.scalar.activation(
                            ex[:, :, qo:QCH], sc[:, :, qo:QCH], ActFn.Exp
                        )
                        pend = (kt, ex, qo)
                    issue_av(*pend)

                    # normalize + ship to the collective input buffer
                    for h in range(2):
                        den = p_sm.tile([1, QCH], F32, name="den", tag="den")
                        nc.vector.tensor_copy(out=den[0:1, :], in_=av[h][0:1, :])
                        rcp = p_sm.tile([1, QCH], F32, name="rcp", tag="rcp")
                        nc.vector.reciprocal_approx_fast(rcp[0:1, :], den[0:1, :])
                        rep = p_sm.tile([128, QCH], F32, name="rep", tag="rep")
                        nc.gpsimd.partition_broadcast(rep[0:128, :], rcp[0:1, :])
                        an = p_sm.tile([128, QCH], BF16, name="an", tag="an")
                        nc.vector.tensor_tensor(
                            out=an[64:128, :],
                            in0=av[h][64:128, :],
                            in1=rep[64:128, :],
                            op=AluOp.mult,
                        )
                        nc.sync.dma_start(
                            out=agin[c][ds(p * 128 + h * 64, 64), :],
                            in_=an[64:128, :],
                        )
                # gather this chunk's attn output across the 4-rank group
                nc.gpsimd.collective_compute(
                    "AllGather",
                    AluOp.bypass,
                    replica_groups=groups,
                    ins=[agin[c].ap()],
                    outs=[agout[c].ap()],
                )

            def outproj_chunk(c):
                pso = [
                    p_ps.tile([128, QCH], F32, name="pso", tag="ps")
                    for _ in range(NPAIR)
                ]
                for e in range(NE):
                    ag_sb = p_ag.tile([128, QCH], BF16, name="ag_sb", tag="ag")
                    nc.sync.dma_start(
                        out=ag_sb[:, :], in_=agout[c][ts(e, 128), :]
                    )
                    for po in range(NPAIR):
                        nc.tensor.matmul(
                            pso[po][:, :],
                            wo_sb[:, e, ts(po, 128)],
                            ag_sb[:, :],
                            start=(e == 0),
                            stop=(e == NE - 1),
                        )
                for po in range(NPAIR):
                    ot = p_out.tile([128, QCH], F32, name="ot", tag="ot")
                    nc.vector.tensor_scalar(
                        out=ot[:, :],
                        in0=pso[po][:, :],
                        scalar1=bo_sb[:, po : po + 1],
                        scalar2=None,
                        op0=AluOp.add,
                    )
                    nc.sync.dma_start(
                        out=out_d[ts(po, 128), ts(c, QCH)], in_=ot[:, :]
                    )

            # --- schedule ---
            # Projection waves ascending (chunk c unlocks attn kt<=4c+3);
            # attention pieces ordered [c1, c2, c3, c0] so the LAST piece is
            # the smallest chunk (short serial tail before the final gather);
            # each out-projection is interleaved one attn chunk later so its
            # gather has landed by the time the PE reaches it.  q(0) is only
            # needed by the final attn piece and is deferred, which lets the
            # exp stream start right after k/v waves 0-1 + q(1).
            with nc.named_scope("wave0"):
                proj_wave(0, 0)
                v_wave(0)
            with nc.named_scope("wave1"):
                proj_wave(0, 1)
                v_wave(1)
                proj_wave(1, 1)
            load_x_wave(2)
            with nc.named_scope("attn1"):
                attn_chunk(1)
            with nc.named_scope("wave2"):
                proj_wave(0, 2)
                v_wave(2)
                proj_wave(1, 2)
            load_x_wave(3)
            load_x_quarter(xq_tiles, xq_t, 0)
            with nc.named_scope("attn2"):
                attn_chunk(2)
            with nc.named_scope("oproj1"):
                outproj_chunk(1)
            with nc.named_scope("wave3"):
                proj_wave(0, 3)
                v_wave(3)
                proj_wave(1, 3)
            with nc.named_scope("attn3"):
                attn_chunk(3)
            with nc.named_scope("oproj2"):
                outproj_chunk(2)
            with nc.named_scope("wave0q"):
                proj_wave(1, 0)
            with nc.named_scope("attn0"):
                attn_chunk(0)
            with nc.named_scope("oproj3"):
                outproj_chunk(3)
            with nc.named_scope("oproj0"):
                outproj_chunk(0)

    nc.compile()
    return nc


_NC_CACHE = None


def _get_nc():
    global _NC_CACHE
    if _NC_CACHE is None:
        _NC_CACHE = build_nc()
    return _NC_CACHE


def _prep_in_maps(query, key, value, Wq, Wk, Wv, Wo, bq, bk, bv, bo, attn_mask):
    query = np.asarray(query, np.float32).reshape(B, S, E)
    key = np.asarray(key, np.float32).reshape(B, S, E)
    value = np.asarray(value, np.float32).reshape(B, S, E)
    m = np.asarray(attn_mask, bool)
    expect = np.triu(np.ones((S, S), bool), k=1)
    if not np.array_equal(m, expect):
        raise ValueError("kernel specialized for causal attn_mask")
    # additive mask for the 4 key-tile offsets inside a diagonal 512-block:
    # maskadd[p, o, f] = NEG where key=128*o+p is masked for query f
    sub = m[:QCH, :QCH]  # [q, k]
    maskadd = np.where(sub.T.reshape(4, 128, QCH), np.float32(NEG), np.float32(0.0))
    maskadd = np.ascontiguousarray(maskadd.transpose(1, 0, 2))  # [128, 4, 512]

    # softmax rows sum to 1, so attn_out = attn @ (x Wv) + bv exactly;
    # fold bv through the out-projection into bo.
    bo_eff = (
        np.asarray(bv, np.float64) @ np.asarray(Wo, np.float64)
        + np.asarray(bo, np.float64)
    ).astype(np.float32)

    xT = [
        [np.ascontiguousarray(t[b].T).astype(NPBF16) for t in (query, key, value)]
        for b in range(B)
    ]

    in_maps = []
    for c in range(NCORES):
        b, g = divmod(c, HG)
        cs = slice(DPC * g, DPC * (g + 1))
        in_maps.append(
            {
                "xq_t": xT[b][0],
                "xk_t": xT[b][1],
                "xv_t": xT[b][2],
                "wq": np.ascontiguousarray(Wq[:, cs]).astype(NPBF16),
                "wk": np.ascontiguousarray(Wk[:, cs]).astype(NPBF16),
                "wv": np.ascontiguousarray(Wv[:, cs]).astype(NPBF16),
                "wo": np.ascontiguousarray(Wo[:, cs]).astype(NPBF16),
                "bq_p": np.ascontiguousarray(
                    np.asarray(bq, np.float32)[cs].reshape(NPAIR, 128).T
                ),
                "bk_p": np.ascontiguousarray(
                    np.asarray(bk, np.float32)[cs].reshape(NPAIR, 128).T
                ),
                "bo_p": np.ascontiguousarray(
                    bo_eff[cs].reshape(NPAIR, 128).T
                ),
                "maskadd": maskadd,
            }
        )
    return in_maps


def _assemble(results):
    outs = []
    for b in range(B):
        cols = [results[b * HG + g]["out_t"] for g in range(HG)]
        outs.append(np.concatenate(cols, axis=0).T)  # [TOK, E]
    return np.ascontiguousarray(np.stack(outs, axis=0).astype(np.float32))


def kernel(**inputs):
    nc = _get_nc()
    in_maps = _prep_in_maps(**inputs)
    res = run_bass_kernel_spmd(nc, in_maps, core_ids=list(range(NCORES)))
    return _assemble(res.results)


if __name__ == "__main__":
    import reference

    inputs = {k: np.asarray(v) for k, v in reference.setup_inputs().items()}
    out = kernel(**inputs)
    exp = np.asarray(reference.reference(**reference.setup_inputs()))
    err = np.abs(out - exp).max() / np.abs(exp).max()
    print("rel err:", err)
